# revision 1
# baseline (speedup 1.0000x reference)
"""BiDirectionalAttention (BiDAF-style) Trainium2 Bass kernel.

Full-input contract: kernel(**inputs) takes the complete unsharded inputs and
returns the full [32, 2048, 512] output. Internally the work is data-parallel
over batch: 8 NeuronCores x 4 batches each.

Per batch b (C=2048 context rows, Q=64 question rows, H=128):
  sim[c,q] = <ctx[c]*w_m, qst[q]> + <w_c, ctx[c]> + <w_q, qst[q]> + mask
  q2c      = softmax_q(sim) @ qst
  c2q      = softmax_c(max_q sim) @ ctx          (one H-vector per batch)
  out      = [ctx | q2c | ctx*q2c | ctx*c2q]     (ctx block assembled on host)

Device layout choices:
  - context is supplied twice: natural [C,H] (elementwise/c2q/output) and
    pre-transposed [H,C] (the sim matmul contracts over H, which must sit on
    the partition dim for the PE).
  - sim is built per 128-row c-tile as PSUM [128, 65]: col 64 carries
    <w_c, ctx[c]> for the second softmax; a K=1 ones-matmul adds the
    question bias row (w_q dot + question_mask) across all partitions.
  - softmax over q is free-dim; softmax over c uses a PE transpose of the
    per-row maxima + a ones-matmul partition reduction.
  - ctx*c2q is computed on the PE as ctxT_tile.T @ diag(c2q).
"""

import os
from contextlib import ExitStack

import numpy as np

import concourse.bacc as bacc
import concourse.mybir as mybir
import concourse.tile as tile
import concourse.bass as bass
from concourse.bass import ts
from concourse.bass_utils import run_bass_kernel_spmd

F32 = mybir.dt.float32
AX = mybir.AxisListType
OP = mybir.AluOpType
AF = mybir.ActivationFunctionType

B, C, Q, H = 32, 2048, 64, 128
NEG = -1e9
NCORES = 8
BP = B // NCORES      # batches per core
TP = 128              # c rows per tile (partition dim)
NT = C // TP          # 16 tiles per batch
WT = 4                # tiles per wave (4 x [128,65] sim fits one PSUM bank)
NW = NT // WT


def build_module(sim_safe=False, repeat=None):
    # sim_safe: CoreSim's matmul visitor asserts result.shape == out_view.shape
    # without flattening free dims, so the wave-wide bias matmul (3D strided
    # out) trips it. The per-tile variant is numerically identical.
    # repeat: wrap the whole workload in a hardware For_i loop (benchmarking
    # only - reruns the same data; output unchanged).
    nc = bacc.Bacc("TRN2", debug=False, num_devices=NCORES)

    ctx_nat = nc.dram_tensor("ctx_nat", [BP, C, H], F32, kind="ExternalInput")
    ctx_t = nc.dram_tensor("ctx_t", [BP, H, C], F32, kind="ExternalInput")
    qst = nc.dram_tensor("qst", [BP, Q, H], F32, kind="ExternalInput")
    rhs_aug = nc.dram_tensor("rhs_aug", [BP, H, Q + 1], F32, kind="ExternalInput")
    bias4 = nc.dram_tensor("bias4", [BP, 1, WT * Q], F32, kind="ExternalInput")
    ident = nc.dram_tensor("ident", [H, H], F32, kind="ExternalInput")
    out = nc.dram_tensor("out", [BP, C, 3 * H], F32, kind="ExternalOutput")

    ctx_nat_ap = ctx_nat.ap().rearrange("b (t p) h -> b p t h", p=TP)
    ctx_t_ap = ctx_t.ap()
    qst_ap = qst.ap()
    rhs_aug_ap = rhs_aug.ap()
    bias4_ap = bias4.ap()
    out_full = out.ap().rearrange("b (t p) j -> b p t j", p=TP)
    out12_ap = out_full[:, :, :, 0 : 2 * H]
    out4_ap = out_full[:, :, :, 2 * H : 3 * H]

    with tile.TileContext(nc) as tc, ExitStack() as ctx:
        const = ctx.enter_context(tc.tile_pool(name="const", bufs=1))
        big = ctx.enter_context(tc.tile_pool(name="big", bufs=2))
        med = ctx.enter_context(tc.tile_pool(name="med", bufs=3))
        small = ctx.enter_context(tc.tile_pool(name="small", bufs=2))
        outp = ctx.enter_context(tc.tile_pool(name="outp", bufs=2))
        ps_sim = ctx.enter_context(tc.tile_pool(name="ps_sim", bufs=4, space="PSUM"))
        ps_et = ctx.enter_context(tc.tile_pool(name="ps_et", bufs=1, space="PSUM"))
        ps_q2c = ctx.enter_context(tc.tile_pool(name="ps_q2c", bufs=2, space="PSUM"))
        ps_misc = ctx.enter_context(tc.tile_pool(name="ps_misc", bufs=1, space="PSUM"))

        ident_sb = const.tile([H, H], F32)
        nc.sync.dma_start(out=ident_sb, in_=ident.ap())
        ones_row = const.tile([1, H], F32)
        nc.vector.memset(ones_row, 1.0)
        ones_col = const.tile([H, 1], F32)
        nc.vector.memset(ones_col, 1.0)

        rep_ctx = tc.For_i(0, repeat, 1) if repeat else None
        if rep_ctx is not None:
            rep_ctx.__enter__()
        for b in range(BP):
            ctxn_sb = big.tile([TP, NT, H], F32, tag="ctxn")
            ctxt_sb = big.tile([H, C], F32, tag="ctxt")
            nc.sync.dma_start(out=ctxt_sb, in_=ctx_t_ap[b])
            nc.sync.dma_start(out=ctxn_sb, in_=ctx_nat_ap[b])
            qst_sb = med.tile([Q, H], F32, tag="qst")
            nc.sync.dma_start(out=qst_sb, in_=qst_ap[b])
            rhsA_sb = med.tile([H, Q + 1], F32, tag="rhs")
            nc.sync.dma_start(out=rhsA_sb, in_=rhs_aug_ap[b])
            bias_sb = med.tile([1, WT * Q], F32, tag="bias")
            nc.sync.dma_start(out=bias_sb, in_=bias4_ap[b])
            bias_w = bias_sb.rearrange("o (k q) -> o k q", k=WT)

            negm = small.tile([TP, NT], F32, tag="negm")
            ssum = small.tile([TP, NT], F32, tag="ssum")
            rall = small.tile([TP, NT], F32, tag="rall")
            rmal = small.tile([TP, NT], F32, tag="rmal")
            stage = outp.tile([TP, NT, 2 * H], F32, tag="stage12")
            stage4 = outp.tile([TP, NT, H], F32, tag="stage4")

            # ---------------- phase 1: sim -> softmax_q -> q2c, per wave ----
            for w in range(NW):
                wsl = slice(w * WT, (w + 1) * WT)
                # The whole wave's sim shares one PSUM bank: a single chained
                # accumulation group (one start, one stop) keeps every write
                # on the lazily-zeroed path.
                sim = ps_sim.tile([TP, WT, Q + 1], F32, tag="sim")
                # (cwc column kept at index Q per tile)
                for k in range(WT):
                    t = w * WT + k
                    nc.tensor.matmul(
                        sim[:, k, :],
                        lhsT=ctxt_sb[:, ts(t, TP)],
                        rhs=rhsA_sb,
                        start=(k == 0),
                        stop=False,
                    )
                # bias row broadcast into all tiles (K=1 rank-1 update)
                if sim_safe:
                    for k in range(WT):
                        nc.tensor.matmul(
                            sim[:, k, 0:Q],
                            lhsT=ones_row,
                            rhs=bias_w[:, k, :],
                            start=False,
                            stop=(k == WT - 1),
                        )
                else:
                    nc.tensor.matmul(
                        sim[:, :, 0:Q],
                        lhsT=ones_row,
                        rhs=bias_w,
                        start=False,
                        stop=True,
                    )

                nc.vector.tensor_reduce(
                    out=negm[:, wsl],
                    in_=sim[:, :, 0:Q],
                    axis=AX.X,
                    op=OP.max,
                    negate=True,
                )
                # shared shift for the whole wave (softmax is shift invariant;
                # per-row max <= wave max keeps exp in (0, 1])
                negm_sh = small.tile([TP, 1], F32, tag="negmsh")
                nc.vector.tensor_reduce(
                    out=negm_sh, in_=negm[:, wsl], axis=AX.X, op=OP.min
                )
                e_sb = med.tile([TP, WT, Q], F32, tag="e")
                nc.scalar.activation(
                    out=e_sb,
                    in_=sim[:, :, 0:Q],
                    func=AF.Exp,
                    bias=negm_sh,
                    scale=1.0,
                )
                nc.vector.tensor_reduce(
                    out=ssum[:, wsl], in_=e_sb, axis=AX.X, op=OP.add
                )
                # row max for the second softmax: rm = cwc - negm
                nc.vector.tensor_sub(rmal[:, wsl], sim[:, :, Q], negm[:, wsl])
                nc.vector.reciprocal(rall[:, wsl], ssum[:, wsl])
                rall_b = bass.AP(
                    tensor=rall.tensor,
                    offset=rall[:, wsl].offset,
                    ap=[rall.ap[0], [rall.ap[1][0], WT], [0, Q]],
                )
                nc.vector.tensor_mul(e_sb, e_sb, rall_b)
                eT_ps = ps_et.tile([Q, WT, TP], F32, tag="eT")
                for k in range(WT):
                    nc.tensor.matmul(
                        eT_ps[:, k, :],
                        lhsT=e_sb[:, k, :],
                        rhs=ident_sb,
                        is_transpose=True,
                        start=(k == 0),
                        stop=(k == WT - 1),
                    )
                eT_sb = med.tile([Q, WT, TP], F32, tag="eTs")
                nc.scalar.copy(out=eT_sb, in_=eT_ps)
                q2c_ps = ps_q2c.tile([TP, WT, H], F32, tag="q2c")
                for k in range(WT):
                    nc.tensor.matmul(
                        q2c_ps[:, k, :],
                        lhsT=eT_sb[:, k, :],
                        rhs=qst_sb,
                        start=(k == 0),
                        stop=(k == WT - 1),
                    )
                nc.scalar.copy(out=stage[:, wsl, 0:H], in_=q2c_ps)
                nc.vector.tensor_mul(
                    stage[:, wsl, H : 2 * H], q2c_ps, ctxn_sb[:, wsl, :]
                )
                # ship this wave's 256 output columns immediately
                nc.sync.dma_start(
                    out=out12_ap[b][:, wsl, :], in_=stage[:, wsl, :]
                )

            # ---------------- phase 2: softmax over c, c2q ------------------
            mx1 = small.tile([TP, 1], F32, tag="mx1")
            nc.vector.tensor_reduce(out=mx1, in_=rmal, axis=AX.X, op=OP.max)
            # [128,1] -> [1,128] so the global max can be reduced on free dim
            mxT_ps = ps_misc.tile([1, TP], F32, tag="ph2s")
            nc.tensor.transpose(mxT_ps, mx1, ident_sb)
            mxT_sb = small.tile([1, TP], F32, tag="mxT")
            nc.vector.tensor_scalar_mul(mxT_sb, mxT_ps, -1.0)
            negM1 = small.tile([1, 1], F32, tag="negM1")
            nc.vector.tensor_reduce(out=negM1, in_=mxT_sb, axis=AX.X, op=OP.min)
            negM_ps = ps_misc.tile([TP, 1], F32, tag="ph2s")
            nc.tensor.matmul(negM_ps, lhsT=ones_row, rhs=negM1, start=True, stop=True)
            negMb = small.tile([TP, 1], F32, tag="negMb")
            nc.vector.tensor_copy(out=negMb, in_=negM_ps)
            exp_rm = small.tile([TP, NT], F32, tag="exprm")
            psums = small.tile([TP, 1], F32, tag="psums")
            nc.scalar.activation(
                out=exp_rm,
                in_=rmal,
                func=AF.Exp,
                bias=negMb,
                scale=1.0,
                accum_out=psums,
            )
            s_ps = ps_misc.tile([1, 1], F32, tag="ph2s")
            nc.tensor.matmul(s_ps, lhsT=psums, rhs=ones_col, start=True, stop=True)
            s_r = small.tile([1, 1], F32, tag="s_r")
            nc.vector.reciprocal(s_r, s_ps)
            c2q_ps = ps_misc.tile([1, H], F32, tag="ph2s")
            for t in range(NT):
                nc.tensor.matmul(
                    c2q_ps,
                    lhsT=exp_rm[:, t : t + 1],
                    rhs=ctxn_sb[:, t, :],
                    start=(t == 0),
                    stop=(t == NT - 1),
                )
            c2q_sb = small.tile([1, H], F32, tag="c2q")
            nc.vector.tensor_scalar_mul(c2q_sb, c2q_ps, s_r)
            c2qb_ps = ps_misc.tile([H, H], F32, tag="ph2s")
            nc.tensor.matmul(c2qb_ps, lhsT=ones_row, rhs=c2q_sb, start=True, stop=True)
            c2qb_sb = small.tile([H, H], F32, tag="c2qb")
            nc.scalar.copy(out=c2qb_sb, in_=c2qb_ps)

            # ---------------- phase 3: ctx * c2q elementwise on DVE ---------
            c2qb_b = bass.AP(
                tensor=c2qb_sb.tensor,
                offset=c2qb_sb.offset,
                ap=[c2qb_sb.ap[0], [0, WT], c2qb_sb.ap[1]],
            )
            for w in range(NW):
                wsl = slice(w * WT, (w + 1) * WT)
                nc.vector.tensor_mul(
                    stage4[:, wsl, :], ctxn_sb[:, wsl, :], c2qb_b
                )
                nc.sync.dma_start(
                    out=out4_ap[b][:, wsl, :], in_=stage4[:, wsl, :]
                )
        if rep_ctx is not None:
            rep_ctx.__exit__(None, None, None)

    nc.compile()
    return nc


_MODULE = None


def _get_module():
    global _MODULE
    if _MODULE is None:
        _MODULE = build_module()
    return _MODULE


def make_in_maps(context, question, question_mask, att_weight):
    """Host-side prep: sharding + layout transforms (no O(B*C*Q*H) compute)."""
    context = np.ascontiguousarray(np.asarray(context, np.float32))
    question = np.ascontiguousarray(np.asarray(question, np.float32))
    qmask = np.asarray(question_mask)
    att_weight = np.asarray(att_weight, np.float32)
    w_c, w_q, w_m = att_weight[:H], att_weight[H : 2 * H], att_weight[2 * H :]

    ctx_t = np.ascontiguousarray(context.transpose(0, 2, 1))
    qmw_t = np.ascontiguousarray((question * w_m[None, None, :]).transpose(0, 2, 1))
    rhs_aug = np.concatenate(
        [qmw_t, np.broadcast_to(w_c[None, :, None], (B, H, 1))], axis=2
    ).astype(np.float32)
    bias = (question @ w_q).astype(np.float32) + np.where(
        qmask, np.float32(0.0), np.float32(NEG)
    ).astype(np.float32)
    bias4 = np.ascontiguousarray(
        np.tile(bias, (1, WT)).reshape(B, 1, WT * Q).astype(np.float32)
    )
    ident = np.eye(H, dtype=np.float32)

    in_maps = []
    for i in range(NCORES):
        sl = slice(i * BP, (i + 1) * BP)
        in_maps.append(
            {
                "ctx_nat": np.ascontiguousarray(context[sl]),
                "ctx_t": np.ascontiguousarray(ctx_t[sl]),
                "qst": np.ascontiguousarray(question[sl]),
                "rhs_aug": np.ascontiguousarray(rhs_aug[sl]),
                "bias4": np.ascontiguousarray(bias4[sl]),
                "ident": ident,
            }
        )
    return in_maps


def assemble_output(context, core_results):
    out = np.empty((B, C, 4 * H), np.float32)
    out[:, :, :H] = context
    for i, res in enumerate(core_results):
        out[i * BP : (i + 1) * BP, :, H:] = res["out"]
    return out


def run(inputs, trace=False, **kwargs):
    context = np.asarray(inputs["context"], np.float32)
    in_maps = make_in_maps(
        context,
        inputs["question"],
        inputs["question_mask"],
        inputs["att_weight"],
    )
    nc = _get_module()
    res = run_bass_kernel_spmd(
        nc, in_maps, core_ids=list(range(NCORES)), trace=trace, **kwargs
    )
    return assemble_output(context, res.results), res


def kernel(**inputs):
    out, _ = run(inputs, trace=False)
    return out



# revision 29
# speedup vs baseline: 1.7186x; 1.7186x over previous
"""BiDirectionalAttention (BiDAF-style) Trainium2 Bass kernel.

Full-input contract: kernel(**inputs) takes the complete unsharded inputs and
returns the full [32, 2048, 512] float32 output. Internally data-parallel over
batch: 8 NeuronCores x 4 batches each.

Per batch b (C=2048 context rows, Q=64 question rows, H=128):
  sim[c,q] = <ctx[c]*w_m, qst[q]> + <w_c, ctx[c]> + <w_q, qst[q]> + mask
  q2c      = softmax_q(sim) @ qst
  c2q      = softmax_c(max_q sim) @ ctx          (one H-vector per batch)
  out      = [ctx | q2c | ctx*q2c | ctx*c2q]     (ctx block assembled on host)

Numerics/layout choices (all aimed at the memory roofline):
  - 16-bit everywhere on the wire: ctx is fed twice, fp16 [H,C] for the sim
    matmul (contraction over H needs H on partitions; fp16 keeps the logit
    error ~4e-3 which the softmax tolerates) and bf16 p-major natural layout
    for the elementwise/c2q work. The 3H output columns are written bf16 and
    upcast on the host.
  - no per-row max subtraction: exp() uses a constant -40 logit shift, which
    cancels in both softmax ratios. bf16's f32-like exponent range absorbs
    e^{sim-40} for |sim|<~85, so softmax_q needs no row max at all, and the
    row max for the c2q path is recovered as max_q(e) (monotonic) by a cheap
    bf16 reduce; exp(rowmax+cwc-80) = max_q(e) * e^{cwc-40}.
  - sim is built per 128-row c-tile as PSUM [128, 4, 65]: col 64 carries
    <w_c, ctx[c]>; a K=1 ones-matmul adds the question bias row (w_q dot +
    question_mask) across all partitions.
  - softmax_q weighted sums run on the PE via a packed transpose: two c-tiles
    of e [128, 2x64] transpose into one [128(q-packed), 128] PSUM tile; the
    row sums s come from N=1 ones-matmuls on the same transposed operand, and
    a single DVE divide per wave normalizes q2c straight out of PSUM.
  - DMA: 6 input DMAs + 1 output DMA per batch, all with >=512B contiguous
    descriptors (inputs are host-packed into DMA-friendly layouts).
"""

import os
from contextlib import ExitStack

import numpy as np

import concourse.bacc as bacc
import concourse.mybir as mybir
import concourse.tile as tile
import concourse.bass as bass
from concourse.bass import ts
from concourse.bass_utils import run_bass_kernel_spmd

F32 = mybir.dt.float32
F16 = mybir.dt.float16
BF16 = mybir.dt.bfloat16
AX = mybir.AxisListType
OP = mybir.AluOpType
AF = mybir.ActivationFunctionType

B, C, Q, H = 32, 2048, 64, 128
NCORES = 8
BP = B // NCORES      # batches per core
TP = 128              # c rows per tile (partition dim)
NT = C // TP          # 16 tiles per batch
WT = 4                # tiles per wave ([128, 4, 65] sim fits one PSUM bank)
NW = NT // WT
SHIFT = -40.0         # uniform logit shift: cancels in softmax ratios,
                      # keeps e^sim inside bf16/f32 range
NEGB = -30000.0       # question-mask bias, fp16-representable


def build_module(sim_safe=False, repeat=None, use_pool=True):
    # sim_safe: CoreSim's matmul visitor asserts result.shape == out_view.shape
    # without flattening free dims, so the wave-wide bias matmul (3D strided
    # out) trips it. The per-tile variant is numerically identical.
    # repeat: wrap the whole workload in a hardware For_i loop (benchmarking
    # only - reruns the same data; output unchanged).
    # use_pool: run the two big elementwise output products on the Pool
    # (gpsimd) engine instead of DVE.
    nc = bacc.Bacc("TRN2", debug=False, num_devices=NCORES)

    ctx_t = nc.dram_tensor("ctx_t", [H, BP, C], F16, kind="ExternalInput")
    ctx_n = nc.dram_tensor("ctx_n", [TP, BP, NT, H], BF16, kind="ExternalInput")
    qstE = nc.dram_tensor("qstE", [Q, BP, H], BF16, kind="ExternalInput")
    rhsA = nc.dram_tensor("rhsA", [H, BP, Q + 1], F16, kind="ExternalInput")
    biasW = nc.dram_tensor("biasW", [1, BP, WT * Q], F16, kind="ExternalInput")
    identH = nc.dram_tensor("identH", [H, H], BF16, kind="ExternalInput")
    out = nc.dram_tensor("out", [BP, C, 3 * H], BF16, kind="ExternalOutput")
    out_ap = out.ap().rearrange("b (t p) j -> b p t j", p=TP)

    with tile.TileContext(nc) as tc, ExitStack() as ctx:
        const = ctx.enter_context(tc.tile_pool(name="const", bufs=1))
        csm = ctx.enter_context(tc.tile_pool(name="csm", bufs=2))
        inp = ctx.enter_context(tc.tile_pool(name="inp", bufs=3))
        ebuf = ctx.enter_context(tc.tile_pool(name="ebuf", bufs=2))
        etbuf = ctx.enter_context(tc.tile_pool(name="etbuf", bufs=3))
        small = ctx.enter_context(tc.tile_pool(name="small", bufs=2))
        outp = ctx.enter_context(tc.tile_pool(name="outp", bufs=2))
        ps_sim = ctx.enter_context(tc.tile_pool(name="ps_sim", bufs=2, space="PSUM"))
        ps_et = ctx.enter_context(tc.tile_pool(name="ps_et", bufs=2, space="PSUM"))
        ps_q2c = ctx.enter_context(tc.tile_pool(name="ps_q2c", bufs=2, space="PSUM"))
        ps_misc = ctx.enter_context(tc.tile_pool(name="ps_misc", bufs=2, space="PSUM"))

        ones_row = const.tile([1, H], F16)
        nc.vector.memset(ones_row, 1.0)
        ones_row_bf = const.tile([1, H], BF16)
        nc.vector.memset(ones_row_bf, 1.0)
        ones_c64 = const.tile([Q, 1], BF16)
        nc.vector.memset(ones_c64, 1.0)
        ones_cTP = const.tile([TP, 1], F32)
        nc.vector.memset(ones_cTP, 1.0)
        shift_col = const.tile([TP, 1], F32)
        nc.vector.memset(shift_col, SHIFT)

        rep_ctx = tc.For_i(0, repeat, 1) if repeat else None
        if rep_ctx is not None:
            rep_ctx.__enter__()

        # small per-core loads first (everything batch 0's first wave needs)
        ident_sb = csm.tile([H, H], BF16, tag="ident")
        nc.sync.dma_start(out=ident_sb, in_=identH.ap())
        qstE_sb = csm.tile([Q, BP, H], BF16, tag="qstE")
        nc.sync.dma_start(out=qstE_sb, in_=qstE.ap())
        rhsA_sb = csm.tile([H, BP, Q + 1], F16, tag="rhsA")
        nc.sync.dma_start(out=rhsA_sb, in_=rhsA.ap())
        bias_sb = csm.tile([1, BP, WT * Q], F16, tag="bias")
        nc.sync.dma_start(out=bias_sb, in_=biasW.ap())

        for b in range(BP):
            # per-batch context loads, pipelined 3 deep by the inp pool
            ctxt_sb = inp.tile([H, C], F16, tag="ctxt")
            nc.sync.dma_start(out=ctxt_sb, in_=ctx_t.ap()[:, b])
            ctxn_sb = inp.tile([TP, NT, H], BF16, tag="ctxn")
            nc.sync.dma_start(out=ctxn_sb, in_=ctx_n.ap()[:, b])

            e_all = ebuf.tile([TP, NT, Q], BF16, tag="e")
            e_cwc = small.tile([TP, NT], BF16, tag="ecwc")
            stage = outp.tile([TP, NT, 3 * H], BF16, tag="stage")
            ssum = small.tile([TP, NT], F32, tag="ssum")
            exp_rm = small.tile([TP, NT], BF16, tag="exprm")
            rm_e = small.tile([TP, NT], BF16, tag="rme")

            # -------- phase 1: sim -> e -> q2c, per wave of 4 c-tiles ------
            for w in range(NW):
                wsl = slice(w * WT, (w + 1) * WT)
                sim = ps_sim.tile([TP, WT, Q + 1], F32, tag="sim")
                for k in range(WT):
                    t = w * WT + k
                    nc.tensor.matmul(
                        sim[:, k, :],
                        lhsT=ctxt_sb[:, ts(t, TP)],
                        rhs=rhsA_sb[:, b, :],
                        start=(k == 0),
                        stop=False,
                    )
                bias_w = bias_sb[:, b, :].rearrange("o (k q) -> o k q", k=WT)
                if sim_safe:
                    for k in range(WT):
                        nc.tensor.matmul(
                            sim[:, k, 0:Q],
                            lhsT=ones_row,
                            rhs=bias_w[:, k, :],
                            start=False,
                            stop=(k == WT - 1),
                        )
                else:
                    nc.tensor.matmul(
                        sim[:, :, 0:Q],
                        lhsT=ones_row,
                        rhs=bias_w,
                        start=False,
                        stop=True,
                    )

                # e = exp(sim - 40), bf16 (covers e^{+-87} at 0.4% rel)
                nc.scalar.activation(
                    out=e_all[:, wsl, :],
                    in_=sim[:, :, 0:Q],
                    func=AF.Exp,
                    bias=shift_col,
                    scale=1.0,
                )
                nc.scalar.activation(
                    out=e_cwc[:, wsl],
                    in_=sim[:, :, Q],
                    func=AF.Exp,
                    bias=shift_col,
                    scale=1.0,
                )

                # transpose each c-tile of e: [TP, Q] -> [Q, TP]
                eT_ps = ps_et.tile([Q, WT, TP], BF16, tag="eT")
                for k in range(WT):
                    nc.tensor.matmul(
                        eT_ps[:, k, :],
                        lhsT=e_all[:, w * WT + k, :],
                        rhs=ident_sb,
                        is_transpose=True,
                        start=(k == 0),
                        stop=(k == WT - 1),
                    )
                eT_sb = etbuf.tile([Q, WT, TP], BF16, tag="eTs")
                nc.scalar.copy(out=eT_sb, in_=eT_ps)

                # q2c numerators + row sums on the PE
                q2c_ps = ps_q2c.tile([TP, WT, H], F32, tag="q2c")
                s4_ps = ps_misc.tile([TP, WT], F32, tag="misc")
                for k in range(WT):
                    lhs = eT_sb[:, k, :]
                    nc.tensor.matmul(
                        q2c_ps[:, k, :],
                        lhsT=lhs,
                        rhs=qstE_sb[:, b, :],
                        start=(k == 0),
                        stop=(k == WT - 1),
                    )
                    nc.tensor.matmul(
                        s4_ps[:, k : k + 1],
                        lhsT=lhs,
                        rhs=ones_c64,
                        start=(k == 0),
                        stop=(k == WT - 1),
                    )
                nc.vector.reciprocal(ssum[:, wsl], s4_ps)
                # q2c normalize straight out of PSUM: one mul per wave
                ss_b = bass.AP(
                    tensor=ssum.tensor,
                    offset=ssum[:, wsl].offset,
                    ap=[ssum.ap[0], [ssum.ap[1][0], WT], [0, H]],
                )
                nc.vector.tensor_mul(stage[:, wsl, 0:H], q2c_ps, ss_b)

            # -------- phase 2: softmax over c, c2q -------------------------
            # exp(rowmax+cwc-80) = max_q(e) * e^{cwc-40}
            nc.vector.tensor_reduce(out=rm_e, in_=e_all, axis=AX.X, op=OP.max)
            nc.vector.tensor_mul(exp_rm, rm_e, e_cwc)
            psums = small.tile([TP, 1], F32, tag="psums")
            nc.vector.tensor_reduce(out=psums, in_=exp_rm, axis=AX.X, op=OP.add)
            s2_ps = ps_misc.tile([1, 1], F32, tag="misc")
            nc.tensor.matmul(s2_ps, lhsT=psums, rhs=ones_cTP, start=True, stop=True)
            s2_r = small.tile([1, 1], F32, tag="s2r")
            nc.vector.reciprocal(s2_r, s2_ps)
            c2q_ps = ps_misc.tile([1, H], F32, tag="misc")
            for t in range(NT):
                nc.tensor.matmul(
                    c2q_ps,
                    lhsT=exp_rm[:, t : t + 1],
                    rhs=ctxn_sb[:, t, :],
                    start=(t == 0),
                    stop=(t == NT - 1),
                )
            c2q_sb = small.tile([1, H], BF16, tag="c2q")
            nc.vector.tensor_scalar_mul(c2q_sb, c2q_ps, s2_r)
            c2qb_ps = ps_misc.tile([H, H], F32, tag="misc")
            nc.tensor.matmul(
                c2qb_ps, lhsT=ones_row_bf, rhs=c2q_sb, start=True, stop=True
            )
            c2qb_sb = small.tile([H, H], BF16, tag="c2qb")
            nc.scalar.copy(out=c2qb_sb, in_=c2qb_ps)

            # -------- phase 3: elementwise outputs, one DMA per batch ------
            mul_eng = nc.gpsimd if use_pool else nc.vector
            mul_eng.tensor_mul(stage[:, :, H : 2 * H], stage[:, :, 0:H], ctxn_sb)
            c2qb_b = bass.AP(
                tensor=c2qb_sb.tensor,
                offset=c2qb_sb.offset,
                ap=[c2qb_sb.ap[0], [0, NT], c2qb_sb.ap[1]],
            )
            mul_eng.tensor_mul(stage[:, :, 2 * H : 3 * H], ctxn_sb, c2qb_b)
            nc.sync.dma_start(out=out_ap[b], in_=stage)
        if rep_ctx is not None:
            rep_ctx.__exit__(None, None, None)

    nc.compile()
    return nc


_MODULE = None


def _get_module():
    global _MODULE
    if _MODULE is None:
        _MODULE = build_module()
    return _MODULE


def make_in_maps(context, question, question_mask, att_weight):
    """Host-side prep: sharding + layout/dtype transforms (no attention math)."""
    context = np.ascontiguousarray(np.asarray(context, np.float32))
    question = np.ascontiguousarray(np.asarray(question, np.float32))
    qmask = np.asarray(question_mask)
    att_weight = np.asarray(att_weight, np.float32)
    w_c, w_q, w_m = att_weight[:H], att_weight[H : 2 * H], att_weight[2 * H :]

    import ml_dtypes

    bf16 = ml_dtypes.bfloat16

    qmw_t = (question * w_m[None, None, :]).transpose(0, 2, 1)  # [B, H, Q]
    rhs_full = np.concatenate(
        [qmw_t, np.broadcast_to(w_c[None, :, None], (B, H, 1))], axis=2
    ).astype(np.float16)  # [B, H, Q+1]
    bias = (question @ w_q) + np.where(qmask, np.float32(0.0), np.float32(NEGB))
    bias4 = np.tile(bias.astype(np.float16), (1, WT))  # [B, WT*Q]
    ident = np.eye(H, dtype=bf16)

    ctx_t_full = context.transpose(2, 0, 1).astype(np.float16)  # [H, B, C]
    ctx_n_full = (
        context.reshape(B, NT, TP, H).transpose(2, 0, 1, 3).astype(bf16)
    )  # [TP, B, NT, H]
    qst_t = question.transpose(1, 0, 2).astype(bf16)  # [Q, B, H]
    rhs_t = rhs_full.transpose(1, 0, 2)  # [H, B, Q+1]

    in_maps = []
    for i in range(NCORES):
        sl = slice(i * BP, (i + 1) * BP)
        in_maps.append(
            {
                "ctx_t": np.ascontiguousarray(ctx_t_full[:, sl]),
                "ctx_n": np.ascontiguousarray(ctx_n_full[:, sl]),
                "qstE": np.ascontiguousarray(qst_t[:, sl]),
                "rhsA": np.ascontiguousarray(rhs_t[:, sl]),
                "biasW": np.ascontiguousarray(bias4[sl][None, :, :]),
                "identH": ident,
            }
        )
    return in_maps


OUT_NAMES = ["out"]


def assemble_core0(context, core_out):
    """Assemble core 0's batches only (for CoreSim checking)."""
    out = np.empty((BP, C, 4 * H), np.float32)
    out[:, :, :H] = np.asarray(context, np.float32)[:BP]
    out[:, :, H:] = core_out["out"].astype(np.float32)
    return out


def assemble_output(context, core_results):
    out = np.empty((B, C, 4 * H), np.float32)
    out[:, :, :H] = np.asarray(context, np.float32)
    for i, res in enumerate(core_results):
        out[i * BP : (i + 1) * BP, :, H:] = res["out"].astype(np.float32)
    return out


def run(inputs, trace=False, **kwargs):
    context = np.asarray(inputs["context"], np.float32)
    in_maps = make_in_maps(
        context,
        inputs["question"],
        inputs["question_mask"],
        inputs["att_weight"],
    )
    nc = _get_module()
    res = run_bass_kernel_spmd(
        nc, in_maps, core_ids=list(range(NCORES)), trace=trace, **kwargs
    )
    return assemble_output(context, res.results), res


def kernel(**inputs):
    out, _ = run(inputs, trace=False)
    return out


# revision 70
# speedup vs baseline: 2.2915x; 1.3334x over previous
"""BiDirectionalAttention (BiDAF-style) Trainium2 Bass kernel.

Full-input contract: kernel(**inputs) takes the complete unsharded inputs and
returns the full [32, 2048, 512] float32 output. Internally data-parallel over
batch: 8 NeuronCores x 4 batches each.

Per batch b (C=2048 context rows, Q=64 question rows, H=128):
  sim[c,q] = <ctx[c]*w_m, qst[q]> + <w_c, ctx[c]> + <w_q, qst[q]> + mask
  q2c      = softmax_q(sim) @ qst
  c2q      = softmax_c(max_q sim) @ ctx          (one H-vector per batch)
  out      = [ctx | q2c | ctx*q2c | ctx*c2q]     (ctx block assembled on host)

Numerics/layout choices (all aimed at the memory roofline):
  - 16-bit everywhere on the wire: ctx is fed twice, fp16 [H,C] for the sim
    matmuls (contraction over H needs H on partitions; fp16 keeps the logit
    error ~4e-3 which the softmax tolerates) and bf16 natural layout for the
    elementwise/c2q work. The 3H output columns are written bf16 in a p-major
    layout (each partition one dense 12KB HBM run) and un-permuted + upcast
    on the host. All per-batch loads are dense HBM blocks (b-major host prep).
  - no per-row max subtraction: exp() uses a constant -40 logit shift, which
    cancels in both softmax ratios; bf16's f32-like exponent range absorbs
    e^{sim-40} for |sim|<~85.
  - sim is computed TWICE on the PE, once per layout consumer:
      * c-major [128, 4, 65] per wave (col 64 = <w_c, ctx>, K=1 ones-matmul
        adds the question bias) -> row max + cwc logits for the c2q softmax;
        one fused Act exp with accumulate yields the c2q weights and sums.
      * q-major [64, 512] per wave (lhsT = w_m*qst) -> the softmax_q weights
        e^T DIRECTLY in the layout the q2c matmul needs; the exact f32
        question bias rides in as the Act exp's per-partition bias column.
    This removes all PE transposes and PSUM->SBUF eT copies.
  - q2c row sums come from N=1 ones-matmuls on the same e^T operand; a single
    DVE reciprocal+mul per wave normalizes q2c straight out of PSUM.
  - DMA: 4 small + 2 context loads per batch (SP queue) + 2 half-batch
    output stores (Act queue), every descriptor a dense >=768B run; the
    benchmark repeat loop uses a staggered semaphore reset (no full
    all-engine barrier between iterations).
"""

import os
from contextlib import ExitStack

import numpy as np

import concourse.bacc as bacc
import concourse.mybir as mybir
import concourse.tile as tile
import concourse.bass as bass
from concourse.bass import ts
from concourse.bass_utils import run_bass_kernel_spmd

F32 = mybir.dt.float32
F16 = mybir.dt.float16
BF16 = mybir.dt.bfloat16
AX = mybir.AxisListType
OP = mybir.AluOpType
AF = mybir.ActivationFunctionType

B, C, Q, H = 32, 2048, 64, 128
NCORES = 8
BP = B // NCORES      # batches per core
TP = 128              # c rows per tile (partition dim)
NT = C // TP          # 16 tiles per batch
WT = 4                # tiles per wave ([128, 4, 65] sim fits one PSUM bank)
NW = NT // WT
SHIFT = -40.0         # uniform logit shift: cancels in softmax ratios,
                      # keeps e^sim inside bf16/f32 range
NEGB = -30000.0       # question-mask bias, fp16-representable


def build_module(
    sim_safe=False,
    repeat=None,
    use_pool=0,
    onchip_ctxn=False,
    staggered=True,
    split3=2,
    out_eng="scalar",
    in_eng="sync",
    inbufs=3,
):
    # sim_safe: CoreSim's matmul visitor asserts result.shape == out_view.shape
    # without flattening free dims, so the wave-wide bias matmul (3D strided
    # out) trips it. The per-tile variant is numerically identical.
    # repeat: wrap the whole workload in a hardware For_i loop (benchmarking
    # only - reruns the same data; output unchanged).
    # use_pool: run the two big elementwise output products on the Pool
    # (gpsimd) engine instead of DVE.
    nc = bacc.Bacc("TRN2", debug=False, num_devices=NCORES)

    # b-major context layouts: every per-batch load is one dense block
    ctx_t = nc.dram_tensor("ctx_t", [BP, H, C], F16, kind="ExternalInput")
    if onchip_ctxn:
        identF = nc.dram_tensor("identF", [H, H], F16, kind="ExternalInput")
    else:
        ctx_n = nc.dram_tensor("ctx_n", [BP, TP, NT, H], BF16, kind="ExternalInput")
    qstE = nc.dram_tensor("qstE", [Q, BP, H], BF16, kind="ExternalInput")
    rhsA = nc.dram_tensor("rhsA", [H, BP, Q + 1], F16, kind="ExternalInput")
    biasW = nc.dram_tensor("biasW", [1, BP, WT * Q], F16, kind="ExternalInput")
    biasC = nc.dram_tensor("biasC", [Q, BP], F32, kind="ExternalInput")
    # p-major output: each partition writes one dense 12KB run per batch;
    # the host un-permutes during assembly
    out = nc.dram_tensor("out", [BP, TP, NT, 3 * H], BF16, kind="ExternalOutput")
    out_ap = out.ap()

    with tile.TileContext(nc) as tc, ExitStack() as ctx:
        const = ctx.enter_context(tc.tile_pool(name="const", bufs=1))
        csm = ctx.enter_context(tc.tile_pool(name="csm", bufs=2))
        inp = ctx.enter_context(tc.tile_pool(name="inp", bufs=inbufs))
        etbuf = ctx.enter_context(tc.tile_pool(name="etbuf", bufs=3))
        small = ctx.enter_context(tc.tile_pool(name="small", bufs=2))
        outp = ctx.enter_context(tc.tile_pool(name="outp", bufs=2))
        ps_sim = ctx.enter_context(tc.tile_pool(name="ps_sim", bufs=2, space="PSUM"))
        ps_simT = ctx.enter_context(tc.tile_pool(name="ps_simT", bufs=2, space="PSUM"))
        ps_q2c = ctx.enter_context(tc.tile_pool(name="ps_q2c", bufs=2, space="PSUM"))
        ps_misc = ctx.enter_context(
            tc.tile_pool(name="ps_misc", bufs=1 if onchip_ctxn else 2, space="PSUM")
        )
        if onchip_ctxn:
            ps_ctxT = ctx.enter_context(
                tc.tile_pool(name="ps_ctxT", bufs=1, space="PSUM")
            )

        ones_row = const.tile([1, H], F16)
        nc.vector.memset(ones_row, 1.0)
        ones_row_bf = const.tile([1, H], BF16)
        nc.vector.memset(ones_row_bf, 1.0)
        ones_c64 = const.tile([Q, 1], BF16)
        nc.vector.memset(ones_c64, 1.0)
        ones_cTP = const.tile([TP, 1], F32)
        nc.vector.memset(ones_cTP, 1.0)
        shift80_col = const.tile([TP, 1], F32)
        nc.vector.memset(shift80_col, 2.0 * SHIFT)

        rep_ctx = (
            tc.For_i(0, repeat, 1, staggered_reset=staggered) if repeat else None
        )
        if rep_ctx is not None:
            rep_ctx.__enter__()

        # small per-core loads first (everything batch 0's first wave needs)
        qstE_sb = csm.tile([Q, BP, H], BF16, tag="qstE")
        nc.sync.dma_start(out=qstE_sb, in_=qstE.ap())
        rhsA_sb = csm.tile([H, BP, Q + 1], F16, tag="rhsA")
        nc.sync.dma_start(out=rhsA_sb, in_=rhsA.ap())
        bias_sb = csm.tile([1, BP, WT * Q], F16, tag="bias")
        nc.sync.dma_start(out=bias_sb, in_=biasW.ap())
        biasC_sb = csm.tile([Q, BP], F32, tag="biasC")
        nc.sync.dma_start(out=biasC_sb, in_=biasC.ap())
        if onchip_ctxn:
            identF_sb = csm.tile([H, H], F16, tag="identF")
            nc.sync.dma_start(out=identF_sb, in_=identF.ap())

        for b in range(BP):
            # per-batch context loads, pipelined 3 deep by the inp pool
            ieng = getattr(nc, in_eng)
            ctxt_sb = inp.tile([H, C], F16, tag="ctxt")
            ieng.dma_start(out=ctxt_sb, in_=ctx_t.ap()[b])
            ctxn_sb = inp.tile([TP, NT, H], BF16, tag="ctxn")
            if not onchip_ctxn:
                ieng.dma_start(out=ctxn_sb, in_=ctx_n.ap()[b])

            stage = outp.tile([TP, NT, 3 * H], BF16, tag="stage")
            ssum = small.tile([TP, NT], F32, tag="ssum")
            exp_rm = small.tile([TP, NT], BF16, tag="exprm")
            rmcw = small.tile([TP, NT], F32, tag="rmcw")

            # -------- phase 1: sim -> e -> q2c, per wave of 4 c-tiles ------
            for w in range(NW):
                wsl = slice(w * WT, (w + 1) * WT)
                sim = ps_sim.tile([TP, WT, Q + 1], F32, tag="sim")
                for k in range(WT):
                    t = w * WT + k
                    nc.tensor.matmul(
                        sim[:, k, :],
                        lhsT=ctxt_sb[:, ts(t, TP)],
                        rhs=rhsA_sb[:, b, :],
                        start=(k == 0),
                        stop=False,
                    )
                bias_w = bias_sb[:, b, :].rearrange("o (k q) -> o k q", k=WT)
                if sim_safe:
                    for k in range(WT):
                        nc.tensor.matmul(
                            sim[:, k, 0:Q],
                            lhsT=ones_row,
                            rhs=bias_w[:, k, :],
                            start=False,
                            stop=(k == WT - 1),
                        )
                else:
                    nc.tensor.matmul(
                        sim[:, :, 0:Q],
                        lhsT=ones_row,
                        rhs=bias_w,
                        start=False,
                        stop=True,
                    )

                # q-major sim for this wave's 512 context rows: one matmul,
                # question bias added exactly (f32) during the exp
                simT = ps_simT.tile([Q, WT * TP], F32, tag="simT")
                nc.tensor.matmul(
                    simT,
                    lhsT=rhsA_sb[:, b, 0:Q],
                    rhs=ctxt_sb[:, w * WT * TP : (w + 1) * WT * TP],
                    start=True,
                    stop=True,
                )
                eT_sb = etbuf.tile([Q, WT * TP], BF16, tag="eTs")
                nc.scalar.activation(
                    out=eT_sb,
                    in_=simT,
                    func=AF.Exp,
                    bias=biasC_sb[:, b : b + 1],
                    scale=1.0,
                )

                # rowmax + cwc logits for the c2q softmax (c-major side)
                nc.vector.tensor_reduce(
                    out=rmcw[:, wsl], in_=sim[:, :, 0:Q], axis=AX.X, op=OP.max
                )
                nc.vector.tensor_add(rmcw[:, wsl], rmcw[:, wsl], sim[:, :, Q])

                if onchip_ctxn:
                    # natural-layout ctx for this wave via PE transposes
                    ctxT_ps = ps_ctxT.tile([TP, WT, H], F16, tag="ctxT")
                    for k in range(WT):
                        nc.tensor.matmul(
                            ctxT_ps[:, k, :],
                            lhsT=ctxt_sb[:, ts(w * WT + k, TP)],
                            rhs=identF_sb,
                            is_transpose=True,
                            start=(k == 0),
                            stop=(k == WT - 1),
                        )
                    if w % 2 == 0:
                        nc.scalar.copy(out=ctxn_sb[:, wsl, :], in_=ctxT_ps)
                    else:
                        nc.vector.tensor_copy(out=ctxn_sb[:, wsl, :], in_=ctxT_ps)

                # q2c numerators + row sums on the PE
                q2c_ps = ps_q2c.tile([TP, WT, H], F32, tag="q2c")
                s4_ps = ps_misc.tile([TP, WT], F32, tag="misc")
                for k in range(WT):
                    lhs = eT_sb[:, ts(k, TP)]
                    nc.tensor.matmul(
                        q2c_ps[:, k, :],
                        lhsT=lhs,
                        rhs=qstE_sb[:, b, :],
                        start=(k == 0),
                        stop=(k == WT - 1),
                    )
                    nc.tensor.matmul(
                        s4_ps[:, k : k + 1],
                        lhsT=lhs,
                        rhs=ones_c64,
                        start=(k == 0),
                        stop=(k == WT - 1),
                    )
                nc.vector.reciprocal(ssum[:, wsl], s4_ps)
                # q2c normalize straight out of PSUM: one mul per wave
                ss_b = bass.AP(
                    tensor=ssum.tensor,
                    offset=ssum[:, wsl].offset,
                    ap=[ssum.ap[0], [ssum.ap[1][0], WT], [0, H]],
                )
                nc.vector.tensor_mul(stage[:, wsl, 0:H], q2c_ps, ss_b)

            # -------- phase 2: softmax over c, c2q -------------------------
            psums = small.tile([TP, 1], F32, tag="psums")
            nc.scalar.activation(
                out=exp_rm,
                in_=rmcw,
                func=AF.Exp,
                bias=shift80_col,
                scale=1.0,
                accum_out=psums,
            )
            s2_ps = ps_misc.tile([1, 1], F32, tag="misc")
            nc.tensor.matmul(s2_ps, lhsT=psums, rhs=ones_cTP, start=True, stop=True)
            s2_r = small.tile([1, 1], F32, tag="s2r")
            nc.vector.reciprocal(s2_r, s2_ps)
            c2q_ps = ps_misc.tile([1, H], F32, tag="misc")
            for t in range(NT):
                nc.tensor.matmul(
                    c2q_ps,
                    lhsT=exp_rm[:, t : t + 1],
                    rhs=ctxn_sb[:, t, :],
                    start=(t == 0),
                    stop=(t == NT - 1),
                )
            c2q_sb = small.tile([1, H], BF16, tag="c2q")
            nc.vector.tensor_scalar_mul(c2q_sb, c2q_ps, s2_r)
            c2qb_ps = ps_misc.tile([H, H], F32, tag="misc")
            nc.tensor.matmul(
                c2qb_ps, lhsT=ones_row_bf, rhs=c2q_sb, start=True, stop=True
            )
            c2qb_sb = small.tile([H, H], BF16, tag="c2qb")
            nc.scalar.copy(out=c2qb_sb, in_=c2qb_ps)

            # -------- phase 3: elementwise outputs, chunked so the output
            # DMA starts before the whole batch's muls finish ---------------
            col2_eng = nc.gpsimd if use_pool >= 2 else nc.vector
            col3_eng = nc.gpsimd if use_pool >= 1 else nc.vector
            oeng = getattr(nc, out_eng)
            hn = NT // split3
            for j in range(split3):
                jsl = slice(j * hn, (j + 1) * hn)
                col2_eng.tensor_mul(
                    stage[:, jsl, H : 2 * H], stage[:, jsl, 0:H], ctxn_sb[:, jsl, :]
                )
                c2qb_b = bass.AP(
                    tensor=c2qb_sb.tensor,
                    offset=c2qb_sb.offset,
                    ap=[c2qb_sb.ap[0], [0, hn], c2qb_sb.ap[1]],
                )
                col3_eng.tensor_mul(
                    stage[:, jsl, 2 * H : 3 * H], ctxn_sb[:, jsl, :], c2qb_b
                )
                oeng.dma_start(out=out_ap[b][:, jsl, :], in_=stage[:, jsl, :])
        if rep_ctx is not None:
            rep_ctx.__exit__(None, None, None)

    nc.compile()
    return nc


_MODULE = None


def _get_module():
    global _MODULE
    if _MODULE is None:
        _MODULE = build_module()
    return _MODULE


def make_in_maps(context, question, question_mask, att_weight):
    """Host-side prep: sharding + layout/dtype transforms (no attention math)."""
    context = np.ascontiguousarray(np.asarray(context, np.float32))
    question = np.ascontiguousarray(np.asarray(question, np.float32))
    qmask = np.asarray(question_mask)
    att_weight = np.asarray(att_weight, np.float32)
    w_c, w_q, w_m = att_weight[:H], att_weight[H : 2 * H], att_weight[2 * H :]

    import ml_dtypes

    bf16 = ml_dtypes.bfloat16

    qmw_t = (question * w_m[None, None, :]).transpose(0, 2, 1)  # [B, H, Q]
    rhs_full = np.concatenate(
        [qmw_t, np.broadcast_to(w_c[None, :, None], (B, H, 1))], axis=2
    ).astype(np.float16)  # [B, H, Q+1]
    bias = (question @ w_q) + np.where(qmask, np.float32(0.0), np.float32(NEGB))
    bias4 = np.tile(bias.astype(np.float16), (1, WT))  # [B, WT*Q]
    bias_col = (bias + np.float32(SHIFT)).T.astype(np.float32)  # [Q, B]
    identf = np.eye(H, dtype=np.float16)

    ctx_t_full = context.transpose(0, 2, 1).astype(np.float16)  # [B, H, C]
    ctx_n_full = (
        context.reshape(B, NT, TP, H).transpose(0, 2, 1, 3).astype(bf16)
    )  # [B, TP, NT, H]
    qst_t = question.transpose(1, 0, 2).astype(bf16)  # [Q, B, H]
    rhs_t = rhs_full.transpose(1, 0, 2)  # [H, B, Q+1]

    in_maps = []
    for i in range(NCORES):
        sl = slice(i * BP, (i + 1) * BP)
        in_maps.append(
            {
                "ctx_t": np.ascontiguousarray(ctx_t_full[sl]),
                "ctx_n": np.ascontiguousarray(ctx_n_full[sl]),
                "qstE": np.ascontiguousarray(qst_t[:, sl]),
                "rhsA": np.ascontiguousarray(rhs_t[:, sl]),
                "biasW": np.ascontiguousarray(bias4[sl][None, :, :]),
                "biasC": np.ascontiguousarray(bias_col[:, sl]),
                "identF": identf,
            }
        )
    return in_maps


OUT_NAMES = ["out"]


def filter_in_maps(nc, in_maps):
    """Drop host-prepared tensors the module variant doesn't declare."""
    names = set()
    for alloc in nc.m.functions[0].allocations:
        if isinstance(alloc, mybir.MemoryLocationSet) and alloc.kind == "ExternalInput":
            names.add(alloc.memorylocations[0].name)
    return [{k: v for k, v in m.items() if k in names} for m in in_maps]


def _unpermute(dev_out):
    """[BP, TP, NT, 3H] p-major device layout -> [BP, C, 3H]."""
    return (
        np.asarray(dev_out)
        .astype(np.float32)
        .transpose(0, 2, 1, 3)
        .reshape(BP, C, 3 * H)
    )


def assemble_core0(context, core_out):
    """Assemble core 0's batches only (for CoreSim checking)."""
    out = np.empty((BP, C, 4 * H), np.float32)
    out[:, :, :H] = np.asarray(context, np.float32)[:BP]
    out[:, :, H:] = _unpermute(core_out["out"])
    return out


def assemble_output(context, core_results):
    out = np.empty((B, C, 4 * H), np.float32)
    out[:, :, :H] = np.asarray(context, np.float32)
    for i, res in enumerate(core_results):
        out[i * BP : (i + 1) * BP, :, H:] = _unpermute(res["out"])
    return out


def run(inputs, trace=False, **kwargs):
    context = np.asarray(inputs["context"], np.float32)
    in_maps = make_in_maps(
        context,
        inputs["question"],
        inputs["question_mask"],
        inputs["att_weight"],
    )
    nc = _get_module()
    res = run_bass_kernel_spmd(
        nc,
        filter_in_maps(nc, in_maps),
        core_ids=list(range(NCORES)),
        trace=trace,
        **kwargs,
    )
    return assemble_output(context, res.results), res


def kernel(**inputs):
    out, _ = run(inputs, trace=False)
    return out
